# revision 19
# baseline (speedup 1.0000x reference)
"""Trainium2 Bass kernel for nn_BatchMAFLayer.

Computes, for a batch of B=4096 inputs and N_BATCH=64 MADE/MAF
distributions, the per-(sample, distribution) log-likelihood:

    xr = x[:, :32]
    h  = relu(xr @ (w1*m1)[n]); h = relu(h @ (w2*m2)[n]); o = h @ (w3*m3)[n]
    shift, ls = o d-major-deinterleaved
    y  = (xr - shift) * exp(-ls)
    ll[b, n] = sum_d(-0.5*y^2 - C - ls)

Sharding: the 64 MADEs are split across 8 NeuronCores (8 per core);
every core sees the full batch and writes its 8 output rows (the host
transposes/concatenates per-core [8, B] blocks into [B, 64]).

Device layout is feature-major ("T-space"): activations are stored as
[features, batch] so each layer is matmul(W.T @ actT) with the weight
stack used directly as the stationary (lhsT) operand — no transposes
between layers. Elementwise op time on ACT/DVE scales with the free
(batch) dim only, so ops are merged along partitions wherever possible:

- MADEs are processed in pairs; each hidden-layer PSUM tile is
  [128, 1024] holding both 128-feature halves of one made, consumed by
  a single relu-copy.
- Layer 3 is eight col-tiled M=32 matmuls arranging the pair's outputs
  as [shift_A|shift_B|ls_A|ls_B] so the whole final stage is one op per
  algebraic step (exp / sub / mul / square / +C) on 64-partition rows.
- The d-reduction matmul accumulates all 4 pairs of a chunk into one
  [8, 512] PSUM tile via a [128, 8] coefficient stack with
  zero-padded columns, giving one PSUM->SBUF copy and one output DMA
  per 512-column chunk.

Matmuls run in bf16 (fp32r caps the PE clock at half rate); the final
reduction matmul and its inputs stay fp32r for summation accuracy, and
the (x - shift) term uses exact f32 x.
"""

import numpy as np

import concourse.bass as bass
from concourse import bacc
import concourse.mybir as mybir
import concourse.tile as tile

F32 = mybir.dt.float32
F32R = mybir.dt.float32r
BF16 = mybir.dt.bfloat16
U8 = mybir.dt.uint8
AFT = mybir.ActivationFunctionType

D = 32
N_BATCH = 64
HID = 256
B = 4096
F = 64
NCORES = 8
NPC = N_BATCH // NCORES  # mades per core
CH = 512                 # batch columns per PSUM tile
NCH = B // CH
HALF_LOG_2PI = 0.5 * float(np.log(2.0 * np.pi))

MM_DT = BF16   # network matmul dtype
LL_DT = F32R   # final-reduction matmul dtype


def build_nc():
    nc = bacc.Bacc("TRN2", target_bir_lowering=False)

    xT_d = nc.dram_tensor("xT", [D, B], F32, kind="ExternalInput")
    w1_d = nc.dram_tensor("w1", [NPC, D, HID], F32, kind="ExternalInput")
    w2_d = nc.dram_tensor("w2", [NPC, HID, HID], F32, kind="ExternalInput")
    w3_d = nc.dram_tensor("w3", [NPC, HID, 2 * D], F32, kind="ExternalInput")
    m1_d = nc.dram_tensor("m1", [NPC, D, HID], U8, kind="ExternalInput")
    m2_d = nc.dram_tensor("m2", [NPC, HID, HID], U8, kind="ExternalInput")
    m3_d = nc.dram_tensor("m3", [NPC, HID, 2 * D], U8, kind="ExternalInput")
    out_d = nc.dram_tensor("out", [NPC, B], F32, kind="ExternalOutput")

    with tile.TileContext(nc) as tc:
        with (
            tc.tile_pool(name="persist", bufs=1) as persist,
            tc.tile_pool(name="stage", bufs=2) as stage,
            tc.tile_pool(name="work", bufs=3) as work,
            tc.tile_pool(name="psum_h", bufs=2, space="PSUM") as psum_h,
            tc.tile_pool(name="psum_o", bufs=2, space="PSUM") as psum_o,
            tc.tile_pool(name="psum_l", bufs=2, space="PSUM") as psum_l,
        ):
            # x[:, :D] transposed: f32 master copy replicated to all four
            # 32-partition groups (row-tiled L1 + pair final stage), plus a
            # bf16 copy for the L1 matmuls.
            xrTf = persist.tile([128, B], F32, tag="xrTf")
            for r in range(4):
                nc.sync.dma_start(out=xrTf[32 * r : 32 * r + 32, :], in_=xT_d[:, :])
            xrT = persist.tile([128, B], MM_DT, tag="xrT")
            nc.vector.tensor_copy(xrT, xrTf)

            # coefficient stack for the chunk d-reduction matmul. Column
            # n reduces made n; each pair's z rows are
            # [y2_A | y2_B | ls'_A | ls'_B] (32 rows each), and columns for
            # the other pairs are zero so all four pairs accumulate into one
            # [8, CH] PSUM tile.
            # block j (cols 8j:8j+8) is pair j's stack: only columns 2j and
            # 2j+1 are non-zero, so pair j's matmul contributes solely to
            # output rows 2j:2j+2 while all pairs accumulate one PSUM tile.
            coeff_f = persist.tile([128, NPC * (NPC // 2)], F32, tag="coeff_f")
            nc.vector.memset(coeff_f, 0.0)
            for j in range(NPC // 2):
                for p in range(2):  # made-within-pair (A=0, B=1)
                    col = NPC * j + 2 * j + p
                    nc.vector.memset(coeff_f[32 * p : 32 * p + 32, col : col + 1], -0.5)
                    nc.vector.memset(
                        coeff_f[64 + 32 * p : 96 + 32 * p, col : col + 1], -1.0
                    )
            coeff = persist.tile([128, NPC * (NPC // 2)], LL_DT, tag="coeff")
            nc.vector.tensor_copy(coeff, coeff_f)

            # per-partition bias columns for ScalarE activations
            zbias = persist.tile([128, 1], F32, tag="zbias")
            nc.vector.memset(zbias, 0.0)

            # ---- weight prep -------------------------------------------------
            # W1 packed 4 mades per [128, HID] tile (made i at partitions 32i).
            W1 = []
            for g in range(NPC // 4):
                w1s = stage.tile([128, HID], F32, tag="w1s")
                nc.sync.dma_start(
                    out=w1s, in_=w1_d[4 * g : 4 * g + 4].rearrange("a p f -> (a p) f")
                )
                m1t = stage.tile([128, HID], U8, tag="m1s")
                nc.sync.dma_start(
                    out=m1t, in_=m1_d[4 * g : 4 * g + 4].rearrange("a p f -> (a p) f")
                )
                m1f = stage.tile([128, HID], F32, tag="m1f")
                nc.scalar.copy(m1f, m1t)
                w1t = persist.tile([128, HID], MM_DT, tag=f"w1_{g}")
                nc.vector.tensor_mul(w1t, w1s, m1f)
                W1.append(w1t)

            # W2 per made: [128, 2*HID]; cols [0:HID] = rows 0:128 (k-half a),
            # cols [HID:] = rows 128:256 (k-half b).
            W2 = []
            M2v = m2_d.rearrange("n (a p) f -> n p a f", a=2)
            for n in range(NPC):
                w2s = stage.tile([128, 2 * HID], F32, tag="w2s")
                nc.sync.dma_start(
                    out=w2s.rearrange("p (a f) -> p a f", a=2),
                    in_=w2_d[n].rearrange("(a p) f -> p a f", a=2),
                )
                m2t = stage.tile([128, 2 * HID], U8, tag="m2s")
                nc.sync.dma_start(
                    out=m2t.rearrange("p (a f) -> p a f", a=2), in_=M2v[n]
                )
                m2f = stage.tile([128, 2 * HID], F32, tag="m2f")
                nc.scalar.copy(m2f, m2t)
                w2t = persist.tile([128, 2 * HID], MM_DT, tag=f"w2_{n}")
                nc.vector.tensor_mul(w2t, w2s, m2f)
                W2.append(w2t)

            # W3 per made: load [128, 128] (k-halves side by side), mask, then
            # de-interleave output columns so each k-half block is
            # [shift cols 0:32 | ls cols 32:64].
            W3 = []
            M3v = m3_d.rearrange("n (a p) f -> n p a f", a=2)
            for n in range(NPC):
                w3s = stage.tile([128, 2 * 2 * D], F32, tag="w3s")
                nc.sync.dma_start(
                    out=w3s.rearrange("p (a f) -> p a f", a=2),
                    in_=w3_d[n].rearrange("(a p) f -> p a f", a=2),
                )
                m3t = stage.tile([128, 2 * 2 * D], U8, tag="m3s")
                nc.sync.dma_start(
                    out=m3t.rearrange("p (a f) -> p a f", a=2), in_=M3v[n]
                )
                m3f = stage.tile([128, 2 * 2 * D], F32, tag="m3f")
                nc.scalar.copy(m3f, m3t)
                w3m = stage.tile([128, 2 * 2 * D], MM_DT, tag="w3m")
                nc.vector.tensor_mul(w3m, w3s, m3f)
                w3r = persist.tile([128, 2 * 2 * D], MM_DT, tag=f"w3_{n}")
                for h in range(2):
                    blk = w3m[:, 64 * h : 64 * h + 64].rearrange(
                        "p (f two) -> p two f", two=2
                    )
                    nc.vector.tensor_copy(w3r[:, 64 * h : 64 * h + 32], blk[:, 0, :])
                    nc.vector.tensor_copy(
                        w3r[:, 64 * h + 32 : 64 * h + 64], blk[:, 1, :]
                    )
                W3.append(w3r)

            # ---- main compute: chunk-outer, made-pairs inner ----------------
            for c in range(NCH):
                cs = slice(c * CH, (c + 1) * CH)
                pll = psum_l.tile([NPC, CH], F32, tag="pll")
                for j in range(NPC // 2):
                    nA, nB = 2 * j, 2 * j + 1
                    gA, iA = nA // 4, nA % 4
                    gB, iB = nB // 4, nB % 4
                    rsA = slice(32 * iA, 32 * iA + 32)
                    rsB = slice(32 * iB, 32 * iB + 32)

                    # L1: K=32 row-tiled; per made a [128, 1024] PSUM tile
                    # holds both 128-feature halves ([:, 0:CH]=a, [:, CH:]=b).
                    ph1A = psum_h.tile([128, 2 * CH], F32, tag="ph")
                    ph1B = psum_h.tile([128, 2 * CH], F32, tag="ph")
                    for mo in (0, 128):
                        nc.tensor.matmul(
                            ph1A[:, mo * 4 : mo * 4 + CH],
                            W1[gA][rsA, mo : mo + 128],
                            xrT[rsA, cs],
                            start=True, stop=True,
                            tile_position=(32 * iA, 0),
                        )
                        nc.tensor.matmul(
                            ph1B[:, mo * 4 : mo * 4 + CH],
                            W1[gB][rsB, mo : mo + 128],
                            xrT[rsB, cs],
                            start=True, stop=True,
                            tile_position=(32 * iB, 0),
                        )
                    h1A = work.tile([128, 2 * CH], MM_DT, tag="h1A")
                    nc.scalar.activation(h1A, ph1A, AFT.Relu, bias=zbias)
                    h1B = work.tile([128, 2 * CH], MM_DT, tag="h1B")
                    nc.vector.tensor_scalar_max(h1B, ph1B, 0.0)

                    # L2: K=256 in two chunks per 128-feature output half.
                    ph2A = psum_h.tile([128, 2 * CH], F32, tag="ph")
                    ph2B = psum_h.tile([128, 2 * CH], F32, tag="ph")
                    for ph2, h1, w2t in (
                        (ph2A, h1A, W2[nA]),
                        (ph2B, h1B, W2[nB]),
                    ):
                        for mo in (0, 128):
                            dst = ph2[:, (mo * 4) : (mo * 4) + CH]
                            nc.tensor.matmul(
                                dst, w2t[:, mo : mo + 128], h1[:, 0:CH],
                                start=True, stop=False,
                            )
                            nc.tensor.matmul(
                                dst, w2t[:, HID + mo : HID + mo + 128],
                                h1[:, CH : 2 * CH],
                                start=False, stop=True,
                            )
                    h2A = work.tile([128, 2 * CH], MM_DT, tag="h2A")
                    nc.scalar.activation(h2A, ph2A, AFT.Relu, bias=zbias)
                    h2B = work.tile([128, 2 * CH], MM_DT, tag="h2B")
                    nc.vector.tensor_scalar_max(h2B, ph2B, 0.0)

                    # L3: eight col-tiled M=32 matmuls; pair output rows are
                    # [shift_A 0:32 | shift_B 32:64 | ls_A 64:96 | ls_B 96:128]
                    po3 = psum_o.tile([128, CH], F32, tag="po")
                    for kh, mo in ((0, 0), (1, 64)):  # k-half, w3r col offset
                        st, sp = kh == 0, kh == 1
                        for h2i, w3r, cg in (
                            (h2A, W3[nA], 0),
                            (h2B, W3[nB], 32),
                        ):
                            rhs = h2i[:, kh * CH : (kh + 1) * CH]
                            nc.tensor.matmul(
                                po3[cg : cg + 32, :],
                                w3r[:, mo : mo + 32],
                                rhs,
                                start=st, stop=sp, skip_group_check=True,
                                tile_position=(0, cg),
                            )
                            nc.tensor.matmul(
                                po3[64 + cg : 96 + cg, :],
                                w3r[:, mo + 32 : mo + 64],
                                rhs,
                                start=st, stop=sp, skip_group_check=True,
                                tile_position=(0, 64 + cg),
                            )

                    # final stage: one op per algebraic step on 64-row blocks
                    e = work.tile([64, CH], F32, tag="e")
                    nc.scalar.activation(
                        e, po3[64:128, :], AFT.Exp, bias=zbias[0:64], scale=-1.0
                    )
                    t = work.tile([64, CH], F32, tag="t")
                    nc.vector.tensor_sub(t, xrTf[0:64, cs], po3[0:64, :])
                    y = work.tile([64, CH], F32, tag="y")
                    nc.vector.tensor_mul(y, t, e)
                    z = work.tile([128, CH], LL_DT, tag="z")
                    # alternate the square between engines to balance load
                    if j % 2 == 0:
                        nc.scalar.activation(
                            z[0:64, :], y, AFT.Square, bias=zbias[0:64]
                        )
                    else:
                        nc.vector.tensor_mul(z[0:64, :], y, y)
                    nc.scalar.activation(z[64:128, :], po3[64:128, :], AFT.Copy)

                    # d-reduction: accumulate this pair into the chunk's
                    # [NPC, CH] tile via the zero-padded coefficient blocks.
                    nc.tensor.matmul(
                        pll, coeff[:, NPC * j : NPC * (j + 1)], z,
                        start=(j == 0), stop=(j == NPC // 2 - 1),
                        skip_group_check=True,
                    )

                # -D*C normalization constant folded into the output copy
                llt = work.tile([NPC, CH], F32, tag="llt")
                if c % 2 == 0:
                    nc.vector.tensor_scalar_add(llt, pll, -D * HALF_LOG_2PI)
                else:
                    nc.scalar.activation(
                        llt, pll, AFT.Copy, bias=-D * HALF_LOG_2PI
                    )
                nc.sync.dma_start(out=out_d[:, cs], in_=llt)

    nc.compile()
    return nc


_NC_CACHE = None
RUN_KWARGS = {}
LAST_RESULT = None


def _get_nc():
    global _NC_CACHE
    if _NC_CACHE is None:
        _NC_CACHE = build_nc()
    return _NC_CACHE


def kernel(x, w1, w2, w3, m1, m2, m3):
    from concourse.bass_utils import run_bass_kernel_spmd

    x = np.asarray(x, dtype=np.float32)
    w1 = np.asarray(w1, dtype=np.float32)
    w2 = np.asarray(w2, dtype=np.float32)
    w3 = np.asarray(w3, dtype=np.float32)
    m1 = np.asarray(m1).astype(np.uint8)
    m2 = np.asarray(m2).astype(np.uint8)
    m3 = np.asarray(m3).astype(np.uint8)

    xT = np.ascontiguousarray(x[:, :D].T)

    in_maps = []
    for k in range(NCORES):
        s = slice(k * NPC, (k + 1) * NPC)
        in_maps.append(
            {
                "xT": xT,
                "w1": np.ascontiguousarray(w1[s]),
                "w2": np.ascontiguousarray(w2[s]),
                "w3": np.ascontiguousarray(w3[s]),
                "m1": np.ascontiguousarray(m1[s]),
                "m2": np.ascontiguousarray(m2[s]),
                "m3": np.ascontiguousarray(m3[s]),
            }
        )

    nc = _get_nc()
    res = run_bass_kernel_spmd(nc, in_maps, list(range(NCORES)), **RUN_KWARGS)
    global LAST_RESULT
    LAST_RESULT = res
    results = res.results
    # per-core output is ll^T [NPC, B]; assemble to [B, N_BATCH]
    return np.concatenate([results[k]["out"].T for k in range(NCORES)], axis=1)


# revision 26
# speedup vs baseline: 1.2434x; 1.2434x over previous
"""Trainium2 Bass kernel for nn_BatchMAFLayer.

Computes, for a batch of B=4096 inputs and N_BATCH=64 MADE/MAF
distributions, the per-(sample, distribution) log-likelihood:

    xr = x[:, :32]
    h  = relu(xr @ (w1*m1)[n]); h = relu(h @ (w2*m2)[n]); o = h @ (w3*m3)[n]
    shift, ls = o d-major-deinterleaved
    y  = (xr - shift) * exp(-ls)
    ll[b, n] = sum_d(-0.5*y^2 - C - ls)

Sharding: the 64 MADEs are split across 8 NeuronCores (8 per core);
every core sees the full batch and writes its 8 output rows (the host
transposes/concatenates per-core [8, B] blocks into [B, 64]).

Device layout is feature-major ("T-space"): activations are stored as
[features, batch] so each layer is matmul(W.T @ actT) with the weight
stack used directly as the stationary (lhsT) operand — no transposes
between layers. Elementwise op time on ACT/DVE scales with the free
(batch) dim only, so ops are merged along partitions wherever possible:

- MADEs are processed in pairs; each hidden-layer PSUM tile is
  [128, 1024] holding both 128-feature halves of one made, consumed by
  a single relu-copy.
- Layer 3 is eight col-tiled M=32 matmuls arranging the pair's outputs
  as [shift_A|shift_B|ls_A|ls_B] so the whole final stage is one op per
  algebraic step (exp / sub / mul / square / +C) on 64-partition rows.
- The d-reduction matmul accumulates all 4 pairs of a chunk into one
  [8, 512] PSUM tile via a [128, 8] coefficient stack with
  zero-padded columns, giving one PSUM->SBUF copy and one output DMA
  per 512-column chunk.

Matmuls run in bf16 (fp32r caps the PE clock at half rate); the final
reduction matmul and its inputs stay fp32r for summation accuracy, and
the (x - shift) term uses exact f32 x.
"""

import numpy as np

import concourse.bass as bass
from concourse import bacc
import concourse.mybir as mybir
import concourse.tile as tile

F32 = mybir.dt.float32
F32R = mybir.dt.float32r
BF16 = mybir.dt.bfloat16
U8 = mybir.dt.uint8
AFT = mybir.ActivationFunctionType

D = 32
N_BATCH = 64
HID = 256
B = 4096
F = 64
NCORES = 8
NPC = N_BATCH // NCORES  # mades per core
CH = 512                 # batch columns per PSUM tile
NCH = B // CH
HALF_LOG_2PI = 0.5 * float(np.log(2.0 * np.pi))

MM_DT = BF16   # network matmul dtype
F16 = mybir.dt.float16
LL_DT = F16    # final-reduction matmul + tail dtype


def build_nc():
    nc = bacc.Bacc("TRN2", target_bir_lowering=False)

    xT_d = nc.dram_tensor("xT", [D, B], F32, kind="ExternalInput")
    w1_d = nc.dram_tensor("w1", [NPC, D, HID], F32, kind="ExternalInput")
    w2_d = nc.dram_tensor("w2", [NPC, HID, HID], F32, kind="ExternalInput")
    w3_d = nc.dram_tensor("w3", [NPC, HID, 2 * D], F32, kind="ExternalInput")
    m1_d = nc.dram_tensor("m1", [NPC, D, HID], U8, kind="ExternalInput")
    m2_d = nc.dram_tensor("m2", [NPC, HID, HID], U8, kind="ExternalInput")
    m3_d = nc.dram_tensor("m3", [NPC, HID, 2 * D], U8, kind="ExternalInput")
    out_d = nc.dram_tensor("out", [NPC, B], F32, kind="ExternalOutput")

    with tile.TileContext(nc) as tc:
        with (
            tc.tile_pool(name="persist", bufs=1) as persist,
            tc.tile_pool(name="stage", bufs=2) as stage,
            tc.tile_pool(name="work", bufs=4) as work,
            tc.tile_pool(name="psum_h", bufs=3, space="PSUM") as psum_h,
            tc.tile_pool(name="psum_o", bufs=1, space="PSUM") as psum_o,
            tc.tile_pool(name="psum_l", bufs=1, space="PSUM") as psum_l,
        ):
            # x[:, :D] transposed: f32 master copy replicated to all four
            # 32-partition groups (row-tiled L1 + pair final stage), plus a
            # bf16 copy for the L1 matmuls.
            xrTf = persist.tile([128, B], F32, tag="xrTf")
            for r in range(4):
                nc.sync.dma_start(out=xrTf[32 * r : 32 * r + 32, :], in_=xT_d[:, :])
            xrT = persist.tile([128, B], MM_DT, tag="xrT")
            nc.vector.tensor_copy(xrT, xrTf)

            # coefficient stack for the chunk d-reduction matmul. Column
            # n reduces made n; each pair's z rows are
            # [y2_A | y2_B | ls'_A | ls'_B] (32 rows each), and columns for
            # the other pairs are zero so all four pairs accumulate into one
            # [8, CH] PSUM tile.
            # block j (cols 8j:8j+8) is pair j's stack: only columns 2j and
            # 2j+1 are non-zero, so pair j's matmul contributes solely to
            # output rows 2j:2j+2 while all pairs accumulate one PSUM tile.
            coeff_f = persist.tile([128, NPC * (NPC // 2)], F32, tag="coeff_f")
            nc.vector.memset(coeff_f, 0.0)
            for j in range(NPC // 2):
                for p in range(2):  # made-within-pair (A=0, B=1)
                    col = NPC * j + 2 * j + p
                    nc.vector.memset(coeff_f[32 * p : 32 * p + 32, col : col + 1], -0.5)
                    nc.vector.memset(
                        coeff_f[64 + 32 * p : 96 + 32 * p, col : col + 1], -1.0
                    )
            coeff = persist.tile([128, NPC * (NPC // 2)], LL_DT, tag="coeff")
            nc.vector.tensor_copy(coeff, coeff_f)

            # per-partition bias columns for ScalarE activations
            zbias = persist.tile([128, 1], F32, tag="zbias")
            nc.vector.memset(zbias, 0.0)

            # ---- weight prep -------------------------------------------------
            # W1 packed 4 mades per [128, HID] tile (made i at partitions 32i).
            W1 = []
            for g in range(NPC // 4):
                w1s = stage.tile([128, HID], F32, tag="w1s")
                nc.sync.dma_start(
                    out=w1s, in_=w1_d[4 * g : 4 * g + 4].rearrange("a p f -> (a p) f")
                )
                m1t = stage.tile([128, HID], U8, tag="m1s")
                nc.sync.dma_start(
                    out=m1t, in_=m1_d[4 * g : 4 * g + 4].rearrange("a p f -> (a p) f")
                )
                m1f = stage.tile([128, HID], F32, tag="m1f")
                nc.gpsimd.tensor_copy(m1f, m1t)
                w1t = persist.tile([128, HID], MM_DT, tag=f"w1_{g}")
                nc.vector.tensor_mul(w1t, w1s, m1f)
                W1.append(w1t)

            # W2 per made: [128, 2*HID]; cols [0:HID] = rows 0:128 (k-half a),
            # cols [HID:] = rows 128:256 (k-half b).
            W2 = []
            M2v = m2_d.rearrange("n (a p) f -> n p a f", a=2)
            for n in range(NPC):
                w2s = stage.tile([128, 2 * HID], F32, tag="w2s")
                nc.sync.dma_start(
                    out=w2s.rearrange("p (a f) -> p a f", a=2),
                    in_=w2_d[n].rearrange("(a p) f -> p a f", a=2),
                )
                m2t = stage.tile([128, 2 * HID], U8, tag="m2s")
                nc.sync.dma_start(
                    out=m2t.rearrange("p (a f) -> p a f", a=2), in_=M2v[n]
                )
                m2f = stage.tile([128, 2 * HID], F32, tag="m2f")
                nc.gpsimd.tensor_copy(m2f, m2t)
                w2t = persist.tile([128, 2 * HID], MM_DT, tag=f"w2_{n}")
                nc.vector.tensor_mul(w2t, w2s, m2f)
                W2.append(w2t)

            # W3 per made: load [128, 128] (k-halves side by side), mask, then
            # de-interleave output columns so each k-half block is
            # [shift cols 0:32 | ls cols 32:64].
            W3 = []
            M3v = m3_d.rearrange("n (a p) f -> n p a f", a=2)
            for n in range(NPC):
                w3s = stage.tile([128, 2 * 2 * D], F32, tag="w3s")
                nc.sync.dma_start(
                    out=w3s.rearrange("p (a f) -> p a f", a=2),
                    in_=w3_d[n].rearrange("(a p) f -> p a f", a=2),
                )
                m3t = stage.tile([128, 2 * 2 * D], U8, tag="m3s")
                nc.sync.dma_start(
                    out=m3t.rearrange("p (a f) -> p a f", a=2), in_=M3v[n]
                )
                m3f = stage.tile([128, 2 * 2 * D], F32, tag="m3f")
                nc.gpsimd.tensor_copy(m3f, m3t)
                w3m = stage.tile([128, 2 * 2 * D], MM_DT, tag="w3m")
                nc.vector.tensor_mul(w3m, w3s, m3f)
                w3r = persist.tile([128, 2 * 2 * D], MM_DT, tag=f"w3_{n}")
                for h in range(2):
                    blk = w3m[:, 64 * h : 64 * h + 64].rearrange(
                        "p (f two) -> p two f", two=2
                    )
                    nc.vector.tensor_copy(w3r[:, 64 * h : 64 * h + 32], blk[:, 0, :])
                    nc.vector.tensor_copy(
                        w3r[:, 64 * h + 32 : 64 * h + 64], blk[:, 1, :]
                    )
                W3.append(w3r)

            # ---- main compute: chunk-outer, made-pairs inner ----------------
            for c in range(NCH):
                cs = slice(c * CH, (c + 1) * CH)
                pll = psum_l.tile([NPC, CH], F32, tag="pll")
                for j in range(NPC // 2):
                    nA, nB = 2 * j, 2 * j + 1
                    gA, iA = nA // 4, nA % 4
                    gB, iB = nB // 4, nB % 4
                    rsA = slice(32 * iA, 32 * iA + 32)
                    rsB = slice(32 * iB, 32 * iB + 32)

                    # L1: K=32 row-tiled; per made a [128, 1024] PSUM tile
                    # holds both 128-feature halves ([:, 0:CH]=a, [:, CH:]=b).
                    ph1A = psum_h.tile([128, 2 * CH], F32, tag="ph")
                    ph1B = psum_h.tile([128, 2 * CH], F32, tag="ph")
                    for mo in (0, 128):
                        nc.tensor.matmul(
                            ph1A[:, mo * 4 : mo * 4 + CH],
                            W1[gA][rsA, mo : mo + 128],
                            xrT[rsA, cs],
                            start=True, stop=True,
                            tile_position=(32 * iA, 0),
                        )
                        nc.tensor.matmul(
                            ph1B[:, mo * 4 : mo * 4 + CH],
                            W1[gB][rsB, mo : mo + 128],
                            xrT[rsB, cs],
                            start=True, stop=True,
                            tile_position=(32 * iB, 0),
                        )
                    h1A = work.tile([128, 2 * CH], MM_DT, tag="h1A")
                    nc.scalar.activation(h1A, ph1A, AFT.Relu, bias=zbias)
                    h1B = work.tile([128, 2 * CH], MM_DT, tag="h1B")
                    nc.vector.tensor_scalar_max(h1B, ph1B, 0.0)

                    # L2: K=256 in two chunks per 128-feature output half.
                    ph2A = psum_h.tile([128, 2 * CH], F32, tag="ph")
                    ph2B = psum_h.tile([128, 2 * CH], F32, tag="ph")
                    for ph2, h1, w2t in (
                        (ph2A, h1A, W2[nA]),
                        (ph2B, h1B, W2[nB]),
                    ):
                        for mo in (0, 128):
                            dst = ph2[:, (mo * 4) : (mo * 4) + CH]
                            nc.tensor.matmul(
                                dst, w2t[:, mo : mo + 128], h1[:, 0:CH],
                                start=True, stop=False,
                            )
                            nc.tensor.matmul(
                                dst, w2t[:, HID + mo : HID + mo + 128],
                                h1[:, CH : 2 * CH],
                                start=False, stop=True,
                            )
                    h2A = work.tile([128, 2 * CH], MM_DT, tag="h2A")
                    nc.scalar.activation(h2A, ph2A, AFT.Relu, bias=zbias)
                    h2B = work.tile([128, 2 * CH], MM_DT, tag="h2B")
                    nc.vector.tensor_scalar_max(h2B, ph2B, 0.0)

                    # L3: eight col-tiled M=32 matmuls; pair output rows are
                    # [shift_A 0:32 | shift_B 32:64 | ls_A 64:96 | ls_B 96:128]
                    po3 = psum_o.tile([128, CH], F32, tag="po")
                    for kh, mo in ((0, 0), (1, 64)):  # k-half, w3r col offset
                        st, sp = kh == 0, kh == 1
                        for h2i, w3r, cg in (
                            (h2A, W3[nA], 0),
                            (h2B, W3[nB], 32),
                        ):
                            rhs = h2i[:, kh * CH : (kh + 1) * CH]
                            nc.tensor.matmul(
                                po3[cg : cg + 32, :],
                                w3r[:, mo : mo + 32],
                                rhs,
                                start=st, stop=sp, skip_group_check=True,
                                tile_position=(0, cg),
                            )
                            nc.tensor.matmul(
                                po3[64 + cg : 96 + cg, :],
                                w3r[:, mo + 32 : mo + 64],
                                rhs,
                                start=st, stop=sp, skip_group_check=True,
                                tile_position=(0, 64 + cg),
                            )

                    # final stage: one op per algebraic step on 64-row blocks
                    # bf16 tail: y and y^2 are all-SBUF 2-byte packed ops,
                    # which unlocks the DVE 2x/4x perf modes.
                    e = work.tile([64, CH], LL_DT, tag="e")
                    nc.scalar.activation(
                        e, po3[64:128, :], AFT.Exp, bias=zbias[0:64], scale=-1.0
                    )
                    t = work.tile([64, CH], LL_DT, tag="t")
                    nc.vector.tensor_sub(t, xrTf[0:64, cs], po3[0:64, :])
                    y = work.tile([64, CH], LL_DT, tag="y")
                    nc.vector.tensor_mul(y, t, e)
                    z = work.tile([128, CH], LL_DT, tag="z")
                    nc.vector.tensor_mul(z[0:64, :], y, y)
                    nc.scalar.activation(z[64:128, :], po3[64:128, :], AFT.Copy)

                    # d-reduction: accumulate this pair into the chunk's
                    # [NPC, CH] tile via the zero-padded coefficient blocks.
                    nc.tensor.matmul(
                        pll, coeff[:, NPC * j : NPC * (j + 1)], z,
                        start=(j == 0), stop=(j == NPC // 2 - 1),
                        skip_group_check=True,
                    )

                # -D*C normalization constant folded into the output copy
                llt = work.tile([NPC, CH], F32, tag="llt")
                if c % 2 == 0:
                    nc.scalar.activation(
                        llt, pll, AFT.Copy, bias=-D * HALF_LOG_2PI
                    )
                else:
                    nc.vector.tensor_scalar_add(llt, pll, -D * HALF_LOG_2PI)
                nc.sync.dma_start(out=out_d[:, cs], in_=llt)

    nc.compile()
    return nc


_NC_CACHE = None
RUN_KWARGS = {}
LAST_RESULT = None


def _get_nc():
    global _NC_CACHE
    if _NC_CACHE is None:
        _NC_CACHE = build_nc()
    return _NC_CACHE


def kernel(x, w1, w2, w3, m1, m2, m3):
    from concourse.bass_utils import run_bass_kernel_spmd

    x = np.asarray(x, dtype=np.float32)
    w1 = np.asarray(w1, dtype=np.float32)
    w2 = np.asarray(w2, dtype=np.float32)
    w3 = np.asarray(w3, dtype=np.float32)
    m1 = np.asarray(m1).astype(np.uint8)
    m2 = np.asarray(m2).astype(np.uint8)
    m3 = np.asarray(m3).astype(np.uint8)

    xT = np.ascontiguousarray(x[:, :D].T)

    in_maps = []
    for k in range(NCORES):
        s = slice(k * NPC, (k + 1) * NPC)
        in_maps.append(
            {
                "xT": xT,
                "w1": np.ascontiguousarray(w1[s]),
                "w2": np.ascontiguousarray(w2[s]),
                "w3": np.ascontiguousarray(w3[s]),
                "m1": np.ascontiguousarray(m1[s]),
                "m2": np.ascontiguousarray(m2[s]),
                "m3": np.ascontiguousarray(m3[s]),
            }
        )

    nc = _get_nc()
    res = run_bass_kernel_spmd(nc, in_maps, list(range(NCORES)), **RUN_KWARGS)
    global LAST_RESULT
    LAST_RESULT = res
    results = res.results
    # per-core output is ll^T [NPC, B]; assemble to [B, N_BATCH]
    return np.concatenate([results[k]["out"].T for k in range(NCORES)], axis=1)
